# revision 1
# baseline (speedup 1.0000x reference)
"""NNConv/KernelNN GNN message passing on 8 Trainium2 NeuronCores.

Strategy (edges sharded by dst-range across 8 cores):
- Host: sort edges by dst, shard contiguous dst ranges per core, build one-hot
  scatter matrices S (bf16) per 128-edge chunk, gather index tables, transposes.
- Phase 1 (device): edge MLP k1->k2->k3 computes per-edge weight matrices
  w2[e, o*32+i] = W_e[i, o] in bf16, materialized to DRAM (memory regime).
- Phase 2 (device, 4 depths): per chunk: indirect-gather h[src], DVE broadcast
  multiply prod = w2 * h, grouped reduce over i -> msg, PE scatter-matmul
  S^T @ msg accumulating per-node-block aggregates in PSUM. Root term rides the
  same path as "self edges" with S = diag(denom); final per-partition scale by
  1/denom in fp32. AllGather of the updated node features between depths.
- Head: fc2/fc3 on own node shard; host concatenates per-core outputs.
"""
import os
import numpy as np
import ml_dtypes

from concourse import bass, bacc, mybir, tile
from concourse import bass_utils
from concourse.masks import make_identity

F32 = mybir.dt.float32
BF16 = mybir.dt.bfloat16
I32 = mybir.dt.int32
BF = ml_dtypes.bfloat16

WN = 32
N_CORES = 8
DEPTH = 4
P = 128


def _prep(inputs):
    """Host preprocessing -> per-core input maps + meta for the program builder."""
    x = np.asarray(inputs["x"], np.float32)
    ei = np.asarray(inputs["edge_index"]).astype(np.int64)
    ea = np.asarray(inputs["edge_attr"], np.float32)
    N, E = x.shape[0], ei.shape[1]
    NPC = N // N_CORES
    NBLK = (NPC + P - 1) // P
    NPAD = NBLK * P

    src, dst = ei[0], ei[1]
    order = np.argsort(dst, kind="stable")
    src_s, dst_s = src[order], dst[order]
    cnt = np.bincount(dst, minlength=N)
    denom = np.maximum(cnt, 1).astype(np.float32)

    core_of = dst_s // NPC
    loc = dst_s - core_of * NPC
    blk = loc // P
    cb = core_of * NBLK + blk
    cb_cnt = np.bincount(cb, minlength=N_CORES * NBLK)
    cb_start = np.concatenate([[0], np.cumsum(cb_cnt)])
    cpb_e = int(np.ceil(cb_cnt.max() / P))  # edge chunks per block
    CPB = cpb_e + 1                          # + self chunk
    NCH = NBLK * CPB
    EPAD = NBLK * cpb_e * P                  # padded edge slots per core

    idxT = np.zeros((N_CORES, P, NCH), np.int32)
    S_all = np.zeros((N_CORES, NCH * P, P), np.float32)
    eaT = np.zeros((N_CORES, 6, EPAD), np.float32)
    xT = np.zeros((N_CORES, 6, NPAD), np.float32)
    invden = np.ones((N_CORES, P, NBLK), np.float32)

    for c in range(N_CORES):
        xT[c, :, :NPC] = x[c * NPC:(c + 1) * NPC].T
        for b in range(NBLK):
            i0, i1 = cb_start[c * NBLK + b], cb_start[c * NBLK + b + 1]
            for ci in range(cpb_e):
                ch = b * CPB + ci
                s0 = i0 + ci * P
                k = max(0, min(P, i1 - s0))
                if k == 0:
                    continue
                sl = slice(s0, s0 + k)
                gsrc = src_s[sl]
                idxT[c, :k, ch] = (gsrc // NPC) * NPAD + gsrc % NPC
                lwb = dst_s[sl] - c * NPC - b * P
                S_all[c, ch * P + np.arange(k), lwb] = 1.0
                esl = (b * cpb_e + ci) * P
                eaT[c, :, esl:esl + k] = ea[order[sl]].T
            # self chunk
            ch = b * CPB + cpb_e
            nb = min(P, NPC - b * P)
            nodes = c * NPC + b * P + np.arange(nb)
            idxT[c, :nb, ch] = c * NPAD + b * P + np.arange(nb)
            S_all[c, ch * P + np.arange(nb), np.arange(nb)] = denom[nodes]
            invden[c, :nb, b] = 1.0 / denom[nodes]

    # weights: (o,i)-permuted k3 / root
    perm = np.arange(WN * WN).reshape(WN, WN).T.flatten()  # (o*32+i) -> i*32+o
    k3w2 = np.asarray(inputs["k3_w"], np.float32)[:, perm]
    k3b2 = np.asarray(inputs["k3_b"], np.float32)[perm][None, :]
    root2 = np.asarray(inputs["root_w"], np.float32).flatten()[perm][None, :]

    shared = {
        "k1w": np.asarray(inputs["k1_w"], np.float32).reshape(6, P),
        "k1b": np.asarray(inputs["k1_b"], np.float32).reshape(P, 1),
        "k2w": np.asarray(inputs["k2_w"], np.float32).astype(BF),
        "k2b": np.asarray(inputs["k2_b"], np.float32).reshape(2, P).T.copy(),
        "k3w2": k3w2.astype(BF),
        "k3b2": k3b2.astype(BF),
        "root2": root2.astype(BF),
        "convb": np.asarray(inputs["conv_b"], np.float32).reshape(1, WN),
        "fc1w": np.asarray(inputs["fc1_w"], np.float32).reshape(6, WN),
        "fc1b": np.asarray(inputs["fc1_b"], np.float32).reshape(1, WN),
        "fc2w": np.asarray(inputs["fc2_w"], np.float32).astype(BF),
        "fc2b": np.asarray(inputs["fc2_b"], np.float32).reshape(1, P),
        "fc3w": np.asarray(inputs["fc3_w"], np.float32).reshape(1, P),
        "fc3b": np.asarray(inputs["fc3_b"], np.float32).reshape(1, 1),
    }
    in_maps = []
    for c in range(N_CORES):
        m = dict(shared)
        m["eaT"] = eaT[c]
        m["xT"] = xT[c]
        m["idxT"] = idxT[c]
        m["S_all"] = S_all[c].astype(BF)
        m["invden"] = invden[c]
        in_maps.append(m)
    meta = dict(N=N, E=E, NPC=NPC, NBLK=NBLK, NPAD=NPAD, cpb_e=cpb_e, CPB=CPB,
                NCH=NCH, EPAD=EPAD)
    return in_maps, meta


def _build(meta):
    NBLK, NPAD, cpb_e, CPB, NCH, EPAD = (meta["NBLK"], meta["NPAD"],
                                         meta["cpb_e"], meta["CPB"],
                                         meta["NCH"], meta["EPAD"])
    HTAB = NPAD * N_CORES
    nc = bacc.Bacc("TRN2", target_bir_lowering=False, debug=False,
                   enable_asserts=False, num_devices=N_CORES)

    def din(name, shape, dt):
        return nc.dram_tensor(name, shape, dt, kind="ExternalInput").ap()

    eaT_d = din("eaT", [6, EPAD], F32)
    xT_d = din("xT", [6, NPAD], F32)
    idxT_d = din("idxT", [P, NCH], I32)
    S_d = din("S_all", [NCH * P, P], BF16)
    invden_d = din("invden", [P, NBLK], F32)
    k1w_d = din("k1w", [6, P], F32)
    k1b_d = din("k1b", [P, 1], F32)
    k2w_d = din("k2w", [P, 256], BF16)
    k2b_d = din("k2b", [P, 2], F32)
    k3w2_d = din("k3w2", [256, WN * WN], BF16)
    k3b2_d = din("k3b2", [1, WN * WN], BF16)
    root2_d = din("root2", [1, WN * WN], BF16)
    convb_d = din("convb", [1, WN], F32)
    fc1w_d = din("fc1w", [6, WN], F32)
    fc1b_d = din("fc1b", [1, WN], F32)
    fc2w_d = din("fc2w", [WN, P], BF16)
    fc2b_d = din("fc2b", [1, P], F32)
    fc3w_d = din("fc3w", [1, P], F32)
    fc3b_d = din("fc3b", [1, 1], F32)
    out_d = nc.dram_tensor("out", [NPAD, 1], F32, kind="ExternalOutput").ap()

    A = mybir.AluOpType
    AF = mybir.ActivationFunctionType

    with tile.TileContext(nc) as tc:
        with tc.tile_pool(name="const", bufs=1) as cp, \
             tc.tile_pool(name="dram", bufs=1, space="DRAM") as dp:
            w2_dram = dp.tile([EPAD, WN * WN], BF16)
            h_own = dp.tile([NPAD, WN], F32)
            h_full = dp.tile([HTAB, WN], F32)

            # resident constants
            idx_t = cp.tile([P, NCH], I32)
            nc.sync.dma_start(idx_t[:], idxT_d[:])
            invd_t = cp.tile([P, NBLK], F32)
            nc.sync.dma_start(invd_t[:], invden_d[:])
            k1w_t = cp.tile([6, P], F32)
            nc.sync.dma_start(k1w_t[:], k1w_d[:])
            k1b_t = cp.tile([P, 1], F32)
            nc.sync.dma_start(k1b_t[:], k1b_d[:])
            k2w_t = cp.tile([P, 256], BF16)
            nc.sync.dma_start(k2w_t[:], k2w_d[:])
            k2b_t = cp.tile([P, 2], F32)
            nc.sync.dma_start(k2b_t[:], k2b_d[:])
            k3a_t = cp.tile([P, WN * WN], BF16)
            nc.sync.dma_start(k3a_t[:], k3w2_d[:P, :])
            k3b_t = cp.tile([P, WN * WN], BF16)
            nc.sync.dma_start(k3b_t[:], k3w2_d[P:, :])
            # partition-broadcast tiles
            k3bias_t = cp.tile([P, WN * WN], BF16)
            nc.sync.dma_start(k3bias_t[:], k3b2_d[:].to_broadcast([P, WN * WN]))
            R_t = cp.tile([P, WN * WN], BF16)
            nc.sync.dma_start(R_t[:], root2_d[:].to_broadcast([P, WN * WN]))
            convb_t = cp.tile([P, WN], F32)
            nc.sync.dma_start(convb_t[:], convb_d[:].to_broadcast([P, WN]))
            fc1w_t = cp.tile([6, WN], F32)
            nc.sync.dma_start(fc1w_t[:], fc1w_d[:])
            fc1b_t = cp.tile([P, WN], F32)
            nc.sync.dma_start(fc1b_t[:], fc1b_d[:].to_broadcast([P, WN]))
            fc2w_t = cp.tile([WN, P], BF16)
            nc.sync.dma_start(fc2w_t[:], fc2w_d[:])
            fc2b_t = cp.tile([P, P], F32)
            nc.sync.dma_start(fc2b_t[:], fc2b_d[:].to_broadcast([P, P]))
            fc3w_t = cp.tile([P, P], F32)
            nc.sync.dma_start(fc3w_t[:], fc3w_d[:].to_broadcast([P, P]))
            fc3b_t = cp.tile([P, 1], F32)
            nc.sync.dma_start(fc3b_t[:], fc3b_d[:].to_broadcast([P, 1]))
            xT_t = cp.tile([6, NPAD], F32)
            nc.sync.dma_start(xT_t[:], xT_d[:])
            ident_t = cp.tile([P, P], F32)
            make_identity(nc, ident_t[:])

            # ---------------- Phase 1: edge MLP -> w2_dram ----------------
            with tc.tile_pool(name="p1", bufs=3) as p1, \
                 tc.tile_pool(name="p1ps", bufs=2, space="PSUM") as pp1, \
                 tc.tile_pool(name="p1ps2", bufs=1, space="PSUM") as pp2:
                for eb in range((EPAD + 511) // 512):
                    ew = min(512, EPAD - eb * 512)
                    ea_t = p1.tile([6, 512], F32, tag="ea")
                    nc.sync.dma_start(ea_t[:, :ew], eaT_d[:, eb * 512:eb * 512 + ew])
                    ps_h1 = pp1.tile([P, 512], F32, tag="h1")
                    nc.tensor.matmul(out=ps_h1[:, :ew], lhsT=k1w_t[:],
                                     rhs=ea_t[:, :ew], start=True, stop=True)
                    h1_t = p1.tile([P, 512], BF16, tag="h1s")
                    nc.scalar.activation(h1_t[:, :ew], ps_h1[:, :ew], AF.Relu,
                                         bias=k1b_t[:, :1])
                    h2t = []
                    for hf in range(2):
                        ps_h2 = pp2.tile([P, 512], F32, tag=f"h2_{hf}")
                        nc.tensor.matmul(out=ps_h2[:, :ew],
                                         lhsT=k2w_t[:, hf * P:(hf + 1) * P],
                                         rhs=h1_t[:, :ew], start=True, stop=True)
                        h2_t = p1.tile([P, 512], BF16, tag=f"h2s_{hf}")
                        nc.scalar.activation(h2_t[:, :ew], ps_h2[:, :ew], AF.Relu,
                                             bias=k2b_t[:, hf:hf + 1])
                        h2t.append(h2_t)
                    for sub in range(ew // P):
                        ps_w = pp1.tile([P, WN * WN], F32, tag="w")
                        sl = slice(sub * P, (sub + 1) * P)
                        for half in range(2):
                            cs = slice(half * 512, (half + 1) * 512)
                            nc.tensor.matmul(out=ps_w[:, cs], lhsT=h2t[0][:, sl],
                                             rhs=k3a_t[:, cs], start=True, stop=False)
                            nc.tensor.matmul(out=ps_w[:, cs], lhsT=h2t[1][:, sl],
                                             rhs=k3b_t[:, cs], start=False, stop=True)
                        w_sb = p1.tile([P, WN * WN], BF16, tag="wsb")
                        nc.scalar.activation(w_sb[:], ps_w[:], AF.Copy)
                        w_sb2 = p1.tile([P, WN * WN], BF16, tag="wsb2")
                        nc.vector.tensor_tensor(out=w_sb2[:], in0=w_sb[:],
                                                in1=k3bias_t[:], op=A.add)
                        r0 = (eb * 4 + sub) * P
                        nc.sync.dma_start(w2_dram[r0:r0 + P, :], w_sb2[:])

            # ---------------- h0 = x @ fc1 + b ----------------
            with tc.tile_pool(name="h0", bufs=2) as hp, \
                 tc.tile_pool(name="h0ps", bufs=2, space="PSUM") as hps:
                for b in range(NBLK):
                    ps = hps.tile([P, WN], F32, tag="h0")
                    nc.tensor.matmul(out=ps[:], lhsT=xT_t[:, b * P:(b + 1) * P],
                                     rhs=fc1w_t[:], start=True, stop=True)
                    h0_t = hp.tile([P, WN], F32, tag="h0s")
                    nc.vector.tensor_tensor(out=h0_t[:], in0=ps[:],
                                            in1=fc1b_t[:, :WN], op=A.add)
                    nc.sync.dma_start(h_own[b * P:(b + 1) * P, :], h0_t[:])
            nc.gpsimd.collective_compute(
                "AllGather", A.bypass,
                replica_groups=[list(range(N_CORES))],
                ins=[h_own.opt()], outs=[h_full.opt()])

            # ---------------- Depth loop ----------------
            for d in range(DEPTH):
                with tc.tile_pool(name=f"d{d}", bufs=3) as dpool, \
                     tc.tile_pool(name=f"d{d}s", bufs=2) as spool, \
                     tc.tile_pool(name=f"d{d}ps", bufs=2, space="PSUM") as dps:
                    for b in range(NBLK):
                        ps_ag = dps.tile([P, WN], F32, tag="aggr")
                        for ci in range(CPB):
                            ch = b * CPB + ci
                            is_self = (ci == cpb_e)
                            h_t = spool.tile([P, WN], F32, tag="hg")
                            nc.gpsimd.indirect_dma_start(
                                out=h_t[:], out_offset=None, in_=h_full[:],
                                in_offset=bass.IndirectOffsetOnAxis(
                                    ap=idx_t[:, ch:ch + 1], axis=0))
                            h_bf = spool.tile([P, WN], BF16, tag="hbf")
                            nc.scalar.activation(h_bf[:], h_t[:], AF.Copy)
                            S_t = spool.tile([P, P], BF16, tag="S")
                            nc.sync.dma_start(S_t[:], S_d[ch * P:(ch + 1) * P, :])
                            if is_self:
                                wsrc = R_t
                            else:
                                wsrc = dpool.tile([P, WN * WN], BF16, tag="w")
                                r0 = (b * cpb_e + ci) * P
                                nc.sync.dma_start(wsrc[:], w2_dram[r0:r0 + P, :])
                            prod = spool.tile([P, WN * WN], BF16, tag="prod")
                            h_b = h_bf[:].rearrange("p (a i) -> p a i", a=1)
                            h_b = h_b.to_broadcast([P, WN, WN])
                            nc.vector.tensor_tensor(
                                out=prod[:].rearrange("p (o i) -> p o i", i=WN),
                                in0=wsrc[:].rearrange("p (o i) -> p o i", i=WN),
                                in1=h_b, op=A.mult)
                            msg = spool.tile([P, WN], F32, tag="msg")
                            nc.vector.tensor_reduce(
                                out=msg[:],
                                in_=prod[:].rearrange("p (o i) -> p o i", i=WN),
                                axis=mybir.AxisListType.X, op=A.add)
                            msg_bf = spool.tile([P, WN], BF16, tag="msgbf")
                            nc.scalar.activation(msg_bf[:], msg[:], AF.Copy)
                            nc.tensor.matmul(out=ps_ag[:], lhsT=S_t[:],
                                             rhs=msg_bf[:], start=(ci == 0),
                                             stop=(ci == CPB - 1))
                        h_pre = spool.tile([P, WN], F32, tag="hpre")
                        nc.scalar.activation(h_pre[:], ps_ag[:], AF.Copy,
                                             scale=invd_t[:, b:b + 1])
                        h_nb = spool.tile([P, WN], F32, tag="hnb")
                        nc.vector.tensor_tensor(out=h_nb[:], in0=h_pre[:],
                                                in1=convb_t[:], op=A.add)
                        if d < DEPTH - 1:
                            h_new = spool.tile([P, WN], F32, tag="hnew")
                            nc.vector.tensor_scalar_max(h_new[:], h_nb[:], 0.0)
                        else:
                            h_new = h_nb
                        nc.sync.dma_start(h_own[b * P:(b + 1) * P, :], h_new[:])
                if d < DEPTH - 1:
                    nc.gpsimd.collective_compute(
                        "AllGather", A.bypass,
                        replica_groups=[list(range(N_CORES))],
                        ins=[h_own.opt()], outs=[h_full.opt()])

            # ---------------- Head: relu(h@fc2+b)@fc3+b ----------------
            with tc.tile_pool(name="hd", bufs=2) as hd, \
                 tc.tile_pool(name="hdps", bufs=2, space="PSUM") as hdp:
                for b in range(NBLK):
                    h_t = hd.tile([P, WN], F32, tag="h")
                    nc.sync.dma_start(h_t[:], h_own[b * P:(b + 1) * P, :])
                    ps_t = hdp.tile([WN, P], F32, tag="tr")
                    nc.tensor.transpose(out=ps_t[:], in_=h_t[:], identity=ident_t[:])
                    hT_bf = hd.tile([WN, P], BF16, tag="hT")
                    nc.scalar.activation(hT_bf[:], ps_t[:], AF.Copy)
                    ps_hh = hdp.tile([P, P], F32, tag="hh")
                    nc.tensor.matmul(out=ps_hh[:], lhsT=hT_bf[:], rhs=fc2w_t[:],
                                     start=True, stop=True)
                    hh1 = hd.tile([P, P], F32, tag="hh1")
                    nc.vector.tensor_tensor(out=hh1[:], in0=ps_hh[:],
                                            in1=fc2b_t[:], op=A.add)
                    hh_bf = hd.tile([P, P], F32, tag="hhbf")
                    nc.vector.tensor_scalar_max(hh_bf[:], hh1[:], 0.0)
                    t3 = hd.tile([P, P], F32, tag="t3")
                    nc.vector.tensor_tensor(out=t3[:], in0=hh_bf[:],
                                            in1=fc3w_t[:], op=A.mult)
                    o1 = hd.tile([P, 1], F32, tag="o1")
                    nc.vector.tensor_reduce(out=o1[:], in_=t3[:],
                                            axis=mybir.AxisListType.X, op=A.add)
                    o2 = hd.tile([P, 1], F32, tag="o2")
                    nc.vector.tensor_tensor(out=o2[:], in0=o1[:],
                                            in1=fc3b_t[:], op=A.add)
                    nc.sync.dma_start(out_d[b * P:(b + 1) * P, :], o2[:])
    nc.compile()
    return nc


def _run_sim(nc, in_maps, meta):
    from concourse.bass_interp import MultiCoreSim
    sim = MultiCoreSim(nc, num_cores=N_CORES, trace=False,
                       require_finite=False, require_nnan=False)
    cores = list(sim.cores.values())
    for c, core in enumerate(cores):
        for k, v in in_maps[c].items():
            core.tensor(k)[:] = v
    sim.simulate(check_with_hw=False)
    return [np.asarray(core.tensor("out")) for core in cores]


def kernel(**inputs):
    in_maps, meta = _prep(inputs)
    nc = _build(meta)
    if os.environ.get("KNN_SIM"):
        outs = _run_sim(nc, in_maps, meta)
    else:
        res = bass_utils.run_bass_kernel_spmd(nc, in_maps, list(range(N_CORES)))
        outs = [res.results[c]["out"] for c in range(N_CORES)]
    NPC = meta["NPC"]
    return np.concatenate([np.asarray(o)[:NPC] for o in outs], axis=0)



# revision 2
# speedup vs baseline: 899.0178x; 899.0178x over previous
"""NNConv/KernelNN GNN message passing on 8 Trainium2 NeuronCores — v5.

vs v4:
- h stored bf16 end-to-end (hstage/h_own/h_full); gathers land directly in the
  mult input dtype (no per-block convert barrier). Numerically identical: every
  consumer already rounded h to bf16.
- w2 DRAM split per block -> phase 1 overlaps depth 0 block-by-block.
- conv bias injected into ps_msg via a denom-row 1xP matmul; epilogue is one
  DVE add + one scalar Relu(scale) write.
- k3 bias added via ones-row matmul in phase 1 (PSUM accumulate), freeing DVE.
- Root-term matmul accumulates ps_msg FIRST so it can run during the AllGather.
"""
import os
import numpy as np
import ml_dtypes

from concourse import bass, bacc, mybir, tile
from concourse import bass_utils
from concourse.masks import make_identity

F32 = mybir.dt.float32
BF16 = mybir.dt.bfloat16
I32 = mybir.dt.int32
BF = ml_dtypes.bfloat16

WN = 32
N_CORES = 8
DEPTH = 4
P = 128

FAT_N = 8      # chunks per block on the PE-fat path (of CPB)


def _prep(inputs):
    x = np.asarray(inputs["x"], np.float32)
    ei = np.asarray(inputs["edge_index"]).astype(np.int64)
    ea = np.asarray(inputs["edge_attr"], np.float32)
    N, E = x.shape[0], ei.shape[1]
    NPC = N // N_CORES
    NBLK = (NPC + P - 1) // P
    NPAD = NBLK * P

    src, dst = ei[0], ei[1]
    cnt = np.bincount(dst, minlength=N)
    denom = np.maximum(cnt, 1).astype(np.float32)

    # balance nodes into blocks per core (greedy LPT on in-degree) so the max
    # edge count per block -- and with it CPB -- is minimized
    pos = np.zeros(N, np.int64)           # node -> c*NPAD + b*P + lane
    node_at = np.full((N_CORES, NPAD), -1, np.int64)
    for c in range(N_CORES):
        deg = cnt[c * NPC:(c + 1) * NPC]
        order = np.argsort(-deg, kind="stable")
        loads = np.zeros(NBLK, np.int64)
        fill = np.zeros(NBLK, np.int64)
        for n in order:
            elig = np.flatnonzero(fill < P)
            b = elig[np.argmin(loads[elig])]
            p = c * NPC + n
            pos[p] = c * NPAD + b * P + fill[b]
            node_at[c, b * P + fill[b]] = p
            loads[b] += deg[n]
            fill[b] += 1

    # sort edges by destination (core, block)
    dstp = pos[dst]
    key = dstp // P                        # global block id (c*NBLK + b)
    order = np.argsort(key, kind="stable")
    src_s, dst_s = src[order], dst[order]
    kb = key[order]
    cb_cnt = np.bincount(kb, minlength=N_CORES * NBLK)
    cb_start = np.concatenate([[0], np.cumsum(cb_cnt)])
    CPB = int(np.ceil(cb_cnt.max() / P))
    NCH = NBLK * CPB
    EPAD = NCH * P

    idxT = np.zeros((N_CORES, P, NCH), np.int32)
    S_all = np.zeros((N_CORES, NCH * P, P), np.float32)
    eaT = np.zeros((N_CORES, 6, EPAD), np.float32)
    xT = np.zeros((N_CORES, 6, NPAD), np.float32)
    invden = np.ones((N_CORES, P, NBLK), np.float32)
    den_row = np.ones((N_CORES, 1, NPAD), np.float32)

    for c in range(N_CORES):
        pp = np.flatnonzero(node_at[c] >= 0)
        nn = node_at[c][pp]
        xT[c][:, pp] = x[nn].T
        invden[c][pp % P, pp // P] = 1.0 / denom[nn]
        den_row[c, 0, pp] = denom[nn]
        for b in range(NBLK):
            i0, i1 = cb_start[c * NBLK + b], cb_start[c * NBLK + b + 1]
            for ci in range(CPB):
                ch = b * CPB + ci
                s0 = i0 + ci * P
                k = max(0, min(P, i1 - s0))
                if k == 0:
                    continue
                sl = slice(s0, s0 + k)
                idxT[c, :k, ch] = pos[src_s[sl]]
                lane = pos[dst_s[sl]] % P
                S_all[c, ch * P + np.arange(k), lane] = 1.0
                eaT[c, :, ch * P:ch * P + k] = ea[order[sl]].T

    perm = np.arange(WN * WN).reshape(WN, WN).T.flatten()  # (o*32+i) -> i*32+o
    k3w2 = np.asarray(inputs["k3_w"], np.float32)[:, perm]
    k3b2 = np.asarray(inputs["k3_b"], np.float32)[perm][None, :]

    shared = {
        "k1w": np.asarray(inputs["k1_w"], np.float32).reshape(6, P),
        "k1b": np.asarray(inputs["k1_b"], np.float32).reshape(P, 1),
        "k2w": np.asarray(inputs["k2_w"], np.float32).astype(BF),
        "k2b": np.asarray(inputs["k2_b"], np.float32).reshape(2, P).T.copy(),
        "k3w2": k3w2.astype(BF),
        "k3b2": k3b2.astype(BF),
        "R32": np.asarray(inputs["root_w"], np.float32).astype(BF),
        "cb32": np.asarray(inputs["conv_b"], np.float32).reshape(1, WN).astype(BF),
        "fc1w": np.asarray(inputs["fc1_w"], np.float32).reshape(6, WN),
        "fc1b": np.asarray(inputs["fc1_b"], np.float32).reshape(1, WN),
        "fc2w": np.asarray(inputs["fc2_w"], np.float32).astype(BF),
        "fc2b": np.asarray(inputs["fc2_b"], np.float32).reshape(1, P),
        "fc3w": np.asarray(inputs["fc3_w"], np.float32).reshape(1, P),
        "fc3b": np.asarray(inputs["fc3_b"], np.float32).reshape(1, 1),
    }
    in_maps = []
    for c in range(N_CORES):
        m = dict(shared)
        m["eaT"] = eaT[c]
        m["xT"] = xT[c]
        m["idxT"] = idxT[c]
        m["S_all"] = S_all[c].astype(BF)
        m["invden"] = invden[c]
        m["den_row"] = den_row[c]
        m["den_bf"] = den_row[c].astype(BF)
        in_maps.append(m)
    meta = dict(N=N, E=E, NPC=NPC, NBLK=NBLK, NPAD=NPAD, CPB=CPB,
                NCH=NCH, EPAD=EPAD, pos=pos)
    return in_maps, meta


def _build(meta):
    NBLK, NPAD, CPB, NCH, EPAD = (meta["NBLK"], meta["NPAD"], meta["CPB"],
                                  meta["NCH"], meta["EPAD"])
    HTAB = NPAD * N_CORES
    W2 = WN * WN
    BPB = CPB * P                       # edge rows per block
    nc = bacc.Bacc("TRN2", target_bir_lowering=False, debug=False,
                   enable_asserts=False, num_devices=N_CORES)

    def din(name, shape, dt):
        return nc.dram_tensor(name, shape, dt, kind="ExternalInput").ap()

    eaT_d = din("eaT", [6, EPAD], F32)
    xT_d = din("xT", [6, NPAD], F32)
    idxT_d = din("idxT", [P, NCH], I32)
    S_d = din("S_all", [NCH * P, P], BF16)
    invden_d = din("invden", [P, NBLK], F32)
    den_d = din("den_row", [1, NPAD], F32)
    denbf_d = din("den_bf", [1, NPAD], BF16)
    k1w_d = din("k1w", [6, P], F32)
    k1b_d = din("k1b", [P, 1], F32)
    k2w_d = din("k2w", [P, 256], BF16)
    k2b_d = din("k2b", [P, 2], F32)
    k3w2_d = din("k3w2", [256, W2], BF16)
    k3b2_d = din("k3b2", [1, W2], BF16)
    R32_d = din("R32", [WN, WN], BF16)
    cb32_d = din("cb32", [1, WN], BF16)
    fc1w_d = din("fc1w", [6, WN], F32)
    fc1b_d = din("fc1b", [1, WN], F32)
    fc2w_d = din("fc2w", [WN, P], BF16)
    fc2b_d = din("fc2b", [1, P], F32)
    fc3w_d = din("fc3w", [1, P], F32)
    fc3b_d = din("fc3b", [1, 1], F32)
    out_d = nc.dram_tensor("out", [NPAD, 1], F32, kind="ExternalOutput").ap()

    A = mybir.AluOpType
    AF = mybir.ActivationFunctionType

    with tile.TileContext(nc) as tc:
        with tc.tile_pool(name="const", bufs=1) as cp, \
             tc.tile_pool(name="dram", bufs=1, space="DRAM") as dp:
            w2_blk = [dp.tile([BPB, W2], BF16, name=f"w2b{b}")
                      for b in range(NBLK)]
            h_own = dp.tile([NPAD, WN], BF16)
            h_full = dp.tile([HTAB, WN], BF16)

            idx_t = cp.tile([P, NCH], I32)
            nc.sync.dma_start(idx_t[:], idxT_d[:])
            invd_t = cp.tile([P, NBLK], F32)
            nc.sync.dma_start(invd_t[:], invden_d[:])
            den_t = cp.tile([WN, NPAD], F32)
            nc.sync.dma_start(den_t[:], den_d[:].to_broadcast([WN, NPAD]))
            denbf_t = cp.tile([1, NPAD], BF16)
            nc.sync.dma_start(denbf_t[:], denbf_d[:])
            k1w_t = cp.tile([6, P], F32)
            nc.sync.dma_start(k1w_t[:], k1w_d[:])
            k1b_t = cp.tile([P, 1], F32)
            nc.sync.dma_start(k1b_t[:], k1b_d[:])
            k2w_t = cp.tile([P, 256], BF16)
            nc.sync.dma_start(k2w_t[:], k2w_d[:])
            k2b_t = cp.tile([P, 2], F32)
            nc.sync.dma_start(k2b_t[:], k2b_d[:])
            k3a_t = cp.tile([P, W2], BF16)
            nc.sync.dma_start(k3a_t[:], k3w2_d[:P, :])
            k3b_t = cp.tile([P, W2], BF16)
            nc.sync.dma_start(k3b_t[:], k3w2_d[P:, :])
            k3bias_t = cp.tile([P, W2], BF16)
            nc.sync.dma_start(k3bias_t[:], k3b2_d[:].to_broadcast([P, W2]))
            R32_t = cp.tile([WN, WN], BF16)
            nc.sync.dma_start(R32_t[:], R32_d[:])
            cb32_t = cp.tile([1, WN], BF16)
            nc.sync.dma_start(cb32_t[:], cb32_d[:])
            fc1w_t = cp.tile([6, WN], F32)
            nc.sync.dma_start(fc1w_t[:], fc1w_d[:])
            fc1b_t = cp.tile([P, WN], F32)
            nc.sync.dma_start(fc1b_t[:], fc1b_d[:].to_broadcast([P, WN]))
            fc2w_t = cp.tile([WN, P], BF16)
            nc.sync.dma_start(fc2w_t[:], fc2w_d[:])
            fc2b_t = cp.tile([P, P], F32)
            nc.sync.dma_start(fc2b_t[:], fc2b_d[:].to_broadcast([P, P]))
            fc3w_t = cp.tile([P, P], F32)
            nc.sync.dma_start(fc3w_t[:], fc3w_d[:].to_broadcast([P, P]))
            fc3b_t = cp.tile([P, 1], F32)
            nc.sync.dma_start(fc3b_t[:], fc3b_d[:].to_broadcast([P, 1]))
            xT_t = cp.tile([6, NPAD], F32)
            nc.sync.dma_start(xT_t[:], xT_d[:])
            identb_t = cp.tile([P, P], BF16)
            make_identity(nc, identb_t[:])
            S_sb = cp.tile([P, NCH * P], BF16)
            nc.scalar.dma_start(
                S_sb[:].rearrange("p (c n) -> p c n", n=P),
                S_d[:].rearrange("(c p) n -> p c n", p=P))
            hstage = cp.tile([P, NBLK * WN], BF16)
            ostage = cp.tile([P, NBLK], F32)

            # ---------------- h0 = x @ fc1 + b (into hstage + h_own) --------
            with tc.tile_pool(name="h0ps", bufs=2, space="PSUM") as hps:
                for b in range(NBLK):
                    ps = hps.tile([P, WN], F32, tag="h0")
                    nc.tensor.matmul(out=ps[:], lhsT=xT_t[:, b * P:(b + 1) * P],
                                     rhs=fc1w_t[:], start=True, stop=True)
                    nc.vector.tensor_tensor(
                        out=hstage[:, b * WN:(b + 1) * WN], in0=ps[:],
                        in1=fc1b_t[:, :WN], op=A.add)
                nc.sync.dma_start(
                    h_own[:].rearrange("(b p) f -> p b f", p=P),
                    hstage[:].rearrange("p (b f) -> p b f", f=WN))
            nc.gpsimd.collective_compute(
                "AllGather", A.bypass,
                replica_groups=[list(range(N_CORES))],
                ins=[h_own.opt()], outs=[h_full.opt()])

            # ---------------- Phase 1: edge MLP -> w2_blk[b] ----------------
            with tc.tile_pool(name="p1", bufs=3) as p1, \
                 tc.tile_pool(name="p1st", bufs=2) as p1s, \
                 tc.tile_pool(name="p1ps", bufs=2, space="PSUM") as pp1, \
                 tc.tile_pool(name="p1ps2", bufs=2, space="PSUM") as pp2:
                for eb in range((EPAD + 511) // 512):
                    e0 = eb * 512
                    ew = min(512, EPAD - e0)
                    ea_t = p1.tile([6, 512], F32, tag="ea")
                    nc.sync.dma_start(ea_t[:, :ew], eaT_d[:, e0:e0 + ew])
                    ps_h1 = pp1.tile([P, 512], F32, tag="h1")
                    nc.tensor.matmul(out=ps_h1[:, :ew], lhsT=k1w_t[:],
                                     rhs=ea_t[:, :ew], start=True, stop=True)
                    h1_t = p1.tile([P, 512], BF16, tag="h1s")
                    nc.scalar.activation(h1_t[:, :ew], ps_h1[:, :ew], AF.Relu,
                                         bias=k1b_t[:, :1])
                    h2t = []
                    for hf in range(2):
                        ps_h2 = pp2.tile([P, 512], F32, tag="h2")
                        nc.tensor.matmul(out=ps_h2[:, :ew],
                                         lhsT=k2w_t[:, hf * P:(hf + 1) * P],
                                         rhs=h1_t[:, :ew], start=True, stop=True)
                        h2_t = p1.tile([P, 512], BF16, tag=f"h2s_{hf}")
                        nc.scalar.activation(h2_t[:, :ew], ps_h2[:, :ew], AF.Relu,
                                             bias=k2b_t[:, hf:hf + 1])
                        h2t.append(h2_t)
                    w_stage = p1s.tile([P, 4 * W2], BF16, tag="wst")
                    for sub in range(ew // P):
                        ps_w = pp1.tile([P, W2], F32, tag="w")
                        sl = slice(sub * P, (sub + 1) * P)
                        for half in range(2):
                            cs = slice(half * 512, (half + 1) * 512)
                            nc.tensor.matmul(out=ps_w[:, cs], lhsT=h2t[0][:, sl],
                                             rhs=k3a_t[:, cs], start=True,
                                             stop=False)
                            nc.tensor.matmul(out=ps_w[:, cs], lhsT=h2t[1][:, sl],
                                             rhs=k3b_t[:, cs], start=False,
                                             stop=True)
                        w_sb = p1.tile([P, W2], BF16, tag="wsb")
                        nc.scalar.activation(w_sb[:], ps_w[:], AF.Copy)
                        nc.vector.tensor_tensor(
                            out=w_stage[:, sub * W2:(sub + 1) * W2],
                            in0=w_sb[:], in1=k3bias_t[:], op=A.add)
                    # store, split at block boundaries
                    n_sub = ew // P
                    s = 0
                    while s < n_sub:
                        r = e0 + s * P
                        b = r // BPB
                        rows = min((b + 1) * BPB - r, (n_sub - s) * P)
                        nsub = rows // P
                        lr = r - b * BPB
                        nc.sync.dma_start(
                            w2_blk[b][lr:lr + rows, :].rearrange(
                                "(s p) f -> p s f", p=P),
                            w_stage[:, s * W2:(s + nsub) * W2].rearrange(
                                "p (s f) -> p s f", f=W2))
                        s += nsub

            # ---------------- Depth loop ----------------
            for d in range(DEPTH):
                with tc.tile_pool(name=f"d{d}", bufs=2) as dpool, \
                     tc.tile_pool(name=f"d{d}p", bufs=4) as ppool, \
                     tc.tile_pool(name=f"d{d}s", bufs=2) as spool, \
                     tc.tile_pool(name=f"d{d}ps", bufs=2, space="PSUM") as dps:
                    for b in range(NBLK):
                        ps_msg = dps.tile([P, WN], F32, tag="msg")
                        # root term first: can run during the AllGather
                        ps_tr = dps.tile([WN, P], BF16, tag="tr")
                        nc.tensor.transpose(
                            out=ps_tr[:], in_=hstage[:, b * WN:(b + 1) * WN],
                            identity=identb_t[:])
                        hT_bf = spool.tile([WN, P], BF16, tag="hT")
                        nc.vector.tensor_tensor(
                            out=hT_bf[:], in0=ps_tr[:],
                            in1=den_t[:, b * P:(b + 1) * P], op=A.mult)
                        nc.tensor.matmul(out=ps_msg[:], lhsT=hT_bf[:],
                                         rhs=R32_t[:], start=True, stop=False)
                        nc.tensor.matmul(out=ps_msg[:],
                                         lhsT=denbf_t[:, b * P:(b + 1) * P],
                                         rhs=cb32_t[:], start=False,
                                         stop=(CPB - FAT_N <= 0))
                        h_bf = spool.tile([P, CPB * WN], BF16, tag="hbf")
                        for ci in range(CPB):
                            nc.gpsimd.indirect_dma_start(
                                out=h_bf[:, ci * WN:(ci + 1) * WN],
                                out_offset=None, in_=h_full[:],
                                in_offset=bass.IndirectOffsetOnAxis(
                                    ap=idx_t[:, b * CPB + ci:b * CPB + ci + 1],
                                    axis=0))
                        w2t = dpool.tile([P, CPB * W2], BF16, tag="w2t")
                        nc.sync.dma_start(
                            w2t[:].rearrange("p (c f) -> p c f", f=W2),
                            w2_blk[b][:].rearrange("(c p) f -> p c f", p=P))
                        ps_ag = dps.tile([P, W2], F32, tag="aggr")
                        nfat = nskin = 0
                        nsk_tot = CPB - FAT_N
                        for ci in range(CPB):
                            ch = b * CPB + ci
                            prod = ppool.tile([P, W2], BF16, tag="prod")
                            h_b = h_bf[:, ci * WN:(ci + 1) * WN].rearrange(
                                "p (a i) -> p a i", a=1).to_broadcast([P, WN, WN])
                            nc.vector.tensor_tensor(
                                out=prod[:].rearrange("p (o i) -> p o i", i=WN),
                                in0=w2t[:, ci * W2:(ci + 1) * W2].rearrange(
                                    "p (o i) -> p o i", i=WN),
                                in1=h_b, op=A.mult)
                            S_ch = S_sb[:, ch * P:(ch + 1) * P]
                            if ci < FAT_N:
                                for half in range(2):
                                    nc.tensor.matmul(
                                        out=ps_ag[:, half * 512:(half + 1) * 512],
                                        lhsT=S_ch,
                                        rhs=prod[:, half * 512:(half + 1) * 512],
                                        start=(nfat == 0),
                                        stop=(nfat == min(FAT_N, CPB) - 1))
                                nfat += 1
                            else:
                                msg = ppool.tile([P, WN], F32, tag="m")
                                nc.vector.tensor_reduce(
                                    out=msg[:],
                                    in_=prod[:].rearrange("p (o i) -> p o i",
                                                          i=WN),
                                    axis=mybir.AxisListType.X, op=A.add)
                                msg_bf = ppool.tile([P, WN], BF16, tag="mb")
                                nc.scalar.activation(msg_bf[:], msg[:], AF.Copy)
                                nskin += 1
                                nc.tensor.matmul(out=ps_msg[:], lhsT=S_ch,
                                                 rhs=msg_bf[:], start=False,
                                                 stop=(nskin == nsk_tot))
                        # epilogue: h_new = act((red + ps_msg) * invden)
                        red = spool.tile([P, WN], F32, tag="red")
                        nc.vector.tensor_reduce(
                            out=red[:],
                            in_=ps_ag[:].rearrange("p (o i) -> p o i", i=WN),
                            axis=mybir.AxisListType.X, op=A.add)
                        t1 = spool.tile([P, WN], F32, tag="t1")
                        nc.vector.tensor_tensor(out=t1[:], in0=red[:],
                                                in1=ps_msg[:], op=A.add)
                        nc.scalar.activation(
                            hstage[:, b * WN:(b + 1) * WN], t1[:],
                            AF.Relu if d < DEPTH - 1 else AF.Copy,
                            scale=invd_t[:, b:b + 1])
                    if d < DEPTH - 1:
                        nc.sync.dma_start(
                            h_own[:].rearrange("(b p) f -> p b f", p=P),
                            hstage[:].rearrange("p (b f) -> p b f", f=WN))
                        nc.gpsimd.collective_compute(
                            "AllGather", A.bypass,
                            replica_groups=[list(range(N_CORES))],
                            ins=[h_own.opt()], outs=[h_full.opt()])

            # ---------------- Head: relu(h@fc2+b)@fc3+b ----------------
            with tc.tile_pool(name="hd", bufs=2) as hd, \
                 tc.tile_pool(name="hdps", bufs=2, space="PSUM") as hdp:
                for b in range(NBLK):
                    ps_t = hdp.tile([WN, P], BF16, tag="tr")
                    nc.tensor.transpose(
                        out=ps_t[:], in_=hstage[:, b * WN:(b + 1) * WN],
                        identity=identb_t[:])
                    hT_bf = hd.tile([WN, P], BF16, tag="hT")
                    nc.scalar.activation(hT_bf[:], ps_t[:], AF.Copy)
                    ps_hh = hdp.tile([P, P], F32, tag="hh")
                    nc.tensor.matmul(out=ps_hh[:], lhsT=hT_bf[:], rhs=fc2w_t[:],
                                     start=True, stop=True)
                    hh1 = hd.tile([P, P], F32, tag="hh1")
                    nc.vector.tensor_tensor(out=hh1[:], in0=ps_hh[:],
                                            in1=fc2b_t[:], op=A.add)
                    hh_r = hd.tile([P, P], F32, tag="hhr")
                    nc.vector.tensor_scalar_max(hh_r[:], hh1[:], 0.0)
                    t3 = hd.tile([P, P], F32, tag="t3")
                    nc.vector.tensor_tensor(out=t3[:], in0=hh_r[:],
                                            in1=fc3w_t[:], op=A.mult)
                    o1 = hd.tile([P, 1], F32, tag="o1")
                    nc.vector.tensor_reduce(out=o1[:], in_=t3[:],
                                            axis=mybir.AxisListType.X, op=A.add)
                    nc.vector.tensor_tensor(out=ostage[:, b:b + 1], in0=o1[:],
                                            in1=fc3b_t[:], op=A.add)
                nc.sync.dma_start(
                    out_d[:].rearrange("(b p) f -> p b f", p=P),
                    ostage[:].rearrange("p (b f) -> p b f", f=1))
    nc.compile()
    return nc


def _run_sim(nc, in_maps, meta):
    from concourse.bass_interp import MultiCoreSim
    sim = MultiCoreSim(nc, num_cores=N_CORES, trace=False,
                       require_finite=False, require_nnan=False)
    cores = list(sim.cores.values())
    for c, core in enumerate(cores):
        for k, v in in_maps[c].items():
            core.tensor(k)[:] = v
    sim.simulate(check_with_hw=False)
    return [np.asarray(core.tensor("out")) for core in cores]


def kernel(**inputs):
    in_maps, meta = _prep(inputs)
    nc = _build(meta)
    if os.environ.get("KNN_SIM"):
        outs = _run_sim(nc, in_maps, meta)
    else:
        res = bass_utils.run_bass_kernel_spmd(nc, in_maps, list(range(N_CORES)))
        outs = [res.results[c]["out"] for c in range(N_CORES)]
    NPAD = meta["NPAD"]
    flat = np.concatenate([np.asarray(o).reshape(NPAD) for o in outs])
    return flat[meta["pos"]][:, None]
